# revision 1
# baseline (speedup 1.0000x reference)
"""v2: layout-B NodeAttention kernel.

Per core: J node-instances. Hidden dim H=256 split into two 128-row halves
living on SBUF/PSUM partitions; nodes live on the free axis. Stats
(sum(h), sum(h*w2), sum(h^2)) are computed by TensorE matmuls against tiny
stationary matrices whose nonzero columns are shifted by 3 per node-tile
("stat-shift"), packing 10 tiles of stats into one PSUM bank. A PE
transpose then flips stats into node-on-partition layout for the cheap
vectorized tail (LN + sigmoid), and gating runs on a node-strided view
of x.

ELU identity used on-device (z' = x@W1 + b1 + 1, bias row baked into the
matmul; e = exp(z'-1)): elu(z) = max(z'-1, min(e,1)-1), stored in fp16.
LayerNorm stats are shift-invariant, so the tail works on either the
shifted or unshifted form.
"""

import sys

for _p in ("/opt/trn_rl_repo", "/root/.axon_site/_ro/trn_rl_repo"):
    if _p not in sys.path:
        sys.path.insert(0, _p)

import contextlib

import numpy as np

import concourse.bacc as bacc
import concourse.bass as bass
import concourse.tile as tile
from concourse import mybir
from concourse.bass_utils import run_bass_kernel_spmd

B = 32
N_NODES = 8192
CPN = 32
HID = 256
LN_EPS = 1e-5

NCORES = 8
BPC = B // NCORES
J = BPC * N_NODES            # 32768 node-instances per core
NTILE = 512                  # nodes per matmul tile
UMAX = 10                    # stat-shift slots per stats bank (3*10 <= 32)

F32 = mybir.dt.float32
F16 = mybir.dt.float16

AT = mybir.ActivationFunctionType
OP = mybir.AluOpType

import os
SQ_ENGINE = os.environ.get("K2_SQ_ENGINE", "gpsimd")
ABLATE = 0
NO_XT_DMA = 0
ELU2OP = int(os.environ.get("K2_ELU2OP", "1"))
MM_DT = os.environ.get("K2_MM_DT", "f32")


def _sq_engine(nc, sq, elu):
    if SQ_ENGINE == "gpsimd":
        nc.gpsimd.tensor_tensor(out=sq, in0=elu, in1=elu, op=OP.mult)
    elif SQ_ENGINE == "vector":
        nc.vector.tensor_tensor(out=sq, in0=elu, in1=elu, op=OP.mult)
    else:
        nc.scalar.activation(sq, elu, AT.Square)


def _stats_ap(trans, ucnt, j, s):
    """View of transposed stats [128, k=4, u=ucnt] for half-block j, slot s."""
    v = trans.rearrange("p (k j m) -> p k j m", k=4, j=4)
    v = v[:, :, j, s:s + 3 * ucnt]
    v = v.rearrange("p k (u s) -> p k u s", s=3)[:, :, :, 0]
    return v.rearrange("p k u -> p u k")


def _node_ap(dram, g, ucnt):
    """Node-strided DRAM view [p=128, k=4, u=ucnt, c=32] for group g."""
    off = g * (UMAX * NTILE) * CPN
    return bass.AP(tensor=dram.tensor, offset=dram.offset + off,
                   ap=[[CPN, 128], [NTILE * CPN, ucnt], [128 * CPN, 4],
                       [1, CPN]])


def _build_program(W1, b1, w2p, s_w2, c_a, j=J, num_devices=NCORES):
    J_, NT = j, j // NTILE
    nc = bacc.Bacc("TRN2", target_bir_lowering=False, debug=False,
                   num_devices=num_devices)

    xt_d = nc.dram_tensor("xt", [CPN + 1, J_], F32, kind="ExternalInput").ap()
    xn_d = nc.dram_tensor("xn", [J_, CPN], F32, kind="ExternalInput").ap()
    w1a_d = nc.dram_tensor("w1a", [CPN + 1, HID], F32,
                           kind="ExternalInput").ap()
    sst_d = nc.dram_tensor("sst", [3, 128, 64], F16, kind="ExternalInput").ap()
    id_d = nc.dram_tensor("ident", [128, 128], F32, kind="ExternalInput").ap()
    out_d = nc.dram_tensor("out", [J_, CPN], F32, kind="ExternalOutput").ap()

    with tile.TileContext(nc) as tc, contextlib.ExitStack() as ctx:
        const = ctx.enter_context(tc.tile_pool(name="const", bufs=1))
        xt_p = ctx.enter_context(tc.tile_pool(name="xtp", bufs=4))
        zp = ctx.enter_context(tc.tile_pool(name="zp", bufs=2, space="PSUM"))
        sp = ctx.enter_context(tc.tile_pool(name="sp", bufs=2, space="PSUM"))
        tp = ctx.enter_context(tc.tile_pool(name="tp", bufs=2, space="PSUM"))
        mid = ctx.enter_context(tc.tile_pool(name="mid", bufs=3))
        tl = ctx.enter_context(tc.tile_pool(name="tl", bufs=2))
        gp = ctx.enter_context(tc.tile_pool(name="gp", bufs=3))

        w1a_s = const.tile([CPN + 1, HID], F32)
        nc.sync.dma_start(out=w1a_s[:], in_=w1a_d[:])
        sa_s = const.tile([128, 64], F16)
        nc.sync.dma_start(out=sa_s[:], in_=sst_d[0])
        sb_s = const.tile([128, 64], F16)
        nc.sync.dma_start(out=sb_s[:], in_=sst_d[1])
        sq_s = const.tile([128, 64], F16)
        nc.sync.dma_start(out=sq_s[:], in_=sst_d[2])
        ident_s = const.tile([128, 128], F32)
        nc.sync.dma_start(out=ident_s[:], in_=id_d[:])
        eps_s = const.tile([128, 1], F32)
        nc.vector.memset(eps_s, LN_EPS)
        neg1_s = const.tile([128, 1], F32)
        nc.vector.memset(neg1_s, -1.0)
        nca_s = const.tile([128, 1], F32)
        nc.vector.memset(nca_s, -c_a)

        stats_ps = None
        elus = {}
        sqs = {}

        def finalize_group(g, ucnt):
            scopy = mid.tile([128, 512], F32, tag="scopy")
            nc.scalar.copy(scopy[:], stats_ps[:])
            trans_ps = tp.tile([128, 512], F32, tag="trans")
            for k in range(4):
                nc.tensor.transpose(trans_ps[:, 128 * k:128 * (k + 1)],
                                    scopy[:, 128 * k:128 * (k + 1)],
                                    ident_s[:])
            trans = tl.tile([128, 512], F32, tag="transs")
            nc.scalar.copy(trans[:], trans_ps[:])
            s1_t = tl.tile([128, UMAX, 4], F32, tag="s1")
            s1 = s1_t[:, :ucnt, :]
            nc.vector.tensor_tensor(out=s1, in0=_stats_ap(trans, ucnt, 0, 0),
                                    in1=_stats_ap(trans, ucnt, 1, 0),
                                    op=OP.add)
            sw_t = tl.tile([128, UMAX, 4], F32, tag="sw")
            sw = sw_t[:, :ucnt, :]
            nc.vector.tensor_tensor(out=sw, in0=_stats_ap(trans, ucnt, 0, 1),
                                    in1=_stats_ap(trans, ucnt, 1, 1),
                                    op=OP.add)
            s2_t = tl.tile([128, UMAX, 4], F32, tag="s2")
            s2 = s2_t[:, :ucnt, :]
            nc.vector.tensor_tensor(out=s2, in0=_stats_ap(trans, ucnt, 2, 2),
                                    in1=_stats_ap(trans, ucnt, 3, 2),
                                    op=OP.add)
            mu_t = tl.tile([128, UMAX, 4], F32, tag="mu")
            mu = mu_t[:, :ucnt, :]
            nc.vector.tensor_scalar_mul(out=mu, in0=s1, scalar1=1.0 / HID)
            msq_t = tl.tile([128, UMAX, 4], F32, tag="msq")
            msq = msq_t[:, :ucnt, :]
            nc.vector.tensor_tensor(out=msq, in0=mu, in1=mu, op=OP.mult)
            var_t = tl.tile([128, UMAX, 4], F32, tag="var")
            var = var_t[:, :ucnt, :]
            nc.vector.scalar_tensor_tensor(out=var, in0=s2, scalar=1.0 / HID,
                                           in1=msq, op0=OP.mult,
                                           op1=OP.subtract)
            sd_t = tl.tile([128, UMAX, 4], F32, tag="sd")
            sd = sd_t[:, :ucnt, :]
            nc.scalar.activation(sd, var, AT.Sqrt, bias=eps_s[:])
            rstd_t = tl.tile([128, UMAX, 4], F32, tag="rstd")
            rstd = rstd_t[:, :ucnt, :]
            nc.vector.reciprocal(rstd, sd)
            n2_t = tl.tile([128, UMAX, 4], F32, tag="n2")
            n2 = n2_t[:, :ucnt, :]
            nc.vector.scalar_tensor_tensor(out=n2, in0=mu, scalar=s_w2,
                                           in1=sw, op0=OP.mult,
                                           op1=OP.subtract)
            n3_t = tl.tile([128, UMAX, 4], F32, tag="n3")
            n3 = n3_t[:, :ucnt, :]
            nc.vector.tensor_tensor(out=n3, in0=n2, in1=rstd, op=OP.mult)
            u1_t = tl.tile([128, UMAX, 4], F32, tag="u1")
            u1 = u1_t[:, :ucnt, :]
            nc.scalar.activation(u1, n3, AT.Exp, bias=nca_s[:])
            vt_t = tl.tile([128, UMAX, 4], F32, tag="vt")
            vt = vt_t[:, :ucnt, :]
            nc.vector.tensor_scalar_add(out=vt, in0=u1, scalar1=1.0)
            gate_t = tl.tile([128, UMAX, 4], F32, tag="gate")
            gate = gate_t[:, :ucnt, :]
            nc.vector.reciprocal(gate, vt)

            xb_t = gp.tile([128, UMAX, 4, CPN], F32, tag="xb")
            xb = xb_t[:, :ucnt, :, :]
            nc.sync.dma_start(out=xb, in_=_node_ap(xn_d, g, ucnt))
            gb = bass.AP(tensor=gate.tensor, offset=gate.offset,
                         ap=list(gate.ap) + [[0, CPN]])
            og_t = gp.tile([128, UMAX, 4, CPN], F32, tag="og")
            og = og_t[:, :ucnt, :, :]
            nc.vector.tensor_tensor(out=og, in0=xb, in1=gb, op=OP.mult)
            nc.sync.dma_start(out=_node_ap(out_d, g, ucnt), in_=og)

        for st in range((NT + 1) // 2):
            tiles = [t for t in (2 * st, 2 * st + 1) if t < NT]
            nt_here = len(tiles)
            xt_t = xt_p.tile([CPN + 1, 2 * NTILE], F32, tag="xt")
            t0 = tiles[0]
            nc.sync.dma_start(
                out=xt_t[:, :nt_here * NTILE],
                in_=xt_d[:, t0 * NTILE:(t0 + nt_here) * NTILE])
            xts = [xt_t[:, i * NTILE:(i + 1) * NTILE]
                   for i in range(nt_here)]
            for half, w_sl in ((0, w1a_s[:, 0:128]), (1, w1a_s[:, 128:256])):
                z = zp.tile([128, 2 * NTILE], F32, tag="z")
                for i, xt_sl in enumerate(xts):
                    nc.tensor.matmul(z[:, i * NTILE:(i + 1) * NTILE],
                                     w_sl, xt_sl, start=True, stop=True)
                zv = z[:, :nt_here * NTILE]
                if ABLATE >= 4:
                    continue
                e_t = mid.tile([128, 2 * NTILE], F16, tag="e")
                e = e_t[:, :nt_here * NTILE]
                nc.scalar.activation(e, zv, AT.Exp, bias=neg1_s[:])
                if ABLATE >= 3:
                    continue
                elu_t = mid.tile([128, 2 * NTILE], F16, tag=f"elu{half}")
                elu = elu_t[:, :nt_here * NTILE]
                if ELU2OP:
                    # unshifted: t1 = min(e,1)-1 (2x fp16), elu = max(z'-1, t1)
                    t1_t = mid.tile([128, 2 * NTILE], F16, tag="t1")
                    t1 = t1_t[:, :nt_here * NTILE]
                    nc.vector.tensor_scalar(out=t1, in0=e, scalar1=1.0,
                                            scalar2=1.0, op0=OP.min,
                                            op1=OP.subtract)
                    nc.vector.scalar_tensor_tensor(out=elu, in0=zv,
                                                   scalar=-1.0, in1=t1,
                                                   op0=OP.add, op1=OP.max)
                else:
                    nc.vector.scalar_tensor_tensor(out=elu, in0=e, scalar=1.0,
                                                   in1=zv, op0=OP.min,
                                                   op1=OP.max)
                sq_t = mid.tile([128, 2 * NTILE], F16, tag=f"sq{half}")
                sq = sq_t[:, :nt_here * NTILE]
                _sq_engine(nc, sq, elu)
                for i, t in enumerate(tiles):
                    elus[(t, half)] = elu[:, i * NTILE:(i + 1) * NTILE]
                    sqs[(t, half)] = sq[:, i * NTILE:(i + 1) * NTILE]

            for i, t in enumerate(tiles):
                if ABLATE >= 2:
                    break
                u = t % UMAX
                if u == 0:
                    stats_ps = sp.tile([128, 512], F32, tag="sps")
                start = u == 0
                stop = u == UMAX - 1 or t == NT - 1
                mm = [(0, sa_s[:, 30 - 3 * u:62 - 3 * u], elus[(t, 0)]),
                      (1, sb_s[:, 30 - 3 * u:62 - 3 * u], elus[(t, 1)]),
                      (2, sq_s[:, 28 - 3 * u:60 - 3 * u], sqs[(t, 0)]),
                      (3, sq_s[:, 28 - 3 * u:60 - 3 * u], sqs[(t, 1)])]
                for jj, lhs, rhs in mm:
                    nc.tensor.matmul(stats_ps[32 * jj:32 * (jj + 1), :],
                                     lhs, rhs, start=start, stop=stop,
                                     tile_position=(0, 32 * jj),
                                     skip_group_check=True)
                del elus[(t, 0)], elus[(t, 1)], sqs[(t, 0)], sqs[(t, 1)]
                if stop and ABLATE < 1:
                    finalize_group(t // UMAX, u + 1)

    nc.compile()
    return nc


def _prep_params(W1, b1, gamma, beta, W2, b2):
    w1a = np.concatenate([W1, (b1 + 1.0)[None, :]], axis=0).astype(np.float32)
    w2p = (W2 * gamma).astype(np.float32)
    s_w2 = float(w2p.sum())
    c_a = float((beta * W2).sum() + b2)
    sst = np.zeros((3, 128, 64), np.float16)
    sst[0, :, 30] = 1.0
    sst[0, :, 31] = w2p[:128]
    sst[1, :, 30] = 1.0
    sst[1, :, 31] = w2p[128:]
    sst[2, :, 30] = 1.0
    ident = np.eye(128, dtype=np.float32)
    return w1a, w2p, s_w2, c_a, sst, ident


def kernel(x, W1, b1, gamma, beta, W2, b2):
    x = np.asarray(x, np.float32)
    w1a, w2p, s_w2, c_a, sst, ident = _prep_params(
        np.asarray(W1, np.float32), np.asarray(b1, np.float32),
        np.asarray(gamma, np.float32), np.asarray(beta, np.float32),
        np.asarray(W2, np.float32), np.asarray(b2, np.float32))

    nc = _build_program(W1, b1, w2p, s_w2, c_a)

    in_maps = []
    for c in range(NCORES):
        xs = x[c * BPC:(c + 1) * BPC].reshape(J, CPN)
        xt = np.empty((CPN + 1, J), np.float32)
        xt[:CPN] = xs.T
        xt[CPN] = 1.0
        in_maps.append({"xt": np.ascontiguousarray(xt),
                        "xn": np.ascontiguousarray(xs),
                        "w1a": w1a, "sst": sst, "ident": ident})

    import os
    trace = bool(int(os.environ.get("BASS_KERNEL_TRACE", "0")))
    res = run_bass_kernel_spmd(nc, in_maps, list(range(NCORES)), trace=trace)
    if trace:
        kernel.last_results = res
    outs = [res.results[c]["out"].reshape(BPC, N_NODES * CPN)
            for c in range(NCORES)]
    return np.concatenate(outs, axis=0)

